# revision 1
# baseline (speedup 1.0000x reference)
"""Trainium2 Bass kernel for nn_ExpSelfAttention (dense transformer block).

Math (per batch item b, all f32 data):
    y  = LN(x; g1, beta1);  z = y @ w_lin.T + b_lin
    attn = W @ z            (W = causal exp-decay matrix, alpha=0.9)
    x2 = x + attn
    y2 = LN(x2; g2, beta2); h = relu(y2 @ w1.T + b1)
    out = x2 + h @ w2.T + b2

Sharding: data parallel over batch (16 / 8 cores = 2 per core); weights and
the (input-independent) decay-matrix blocks replicated. No collectives.

Kernel strategy per core:
  - LN gains folded into the (pre-transposed) weights, LN betas into bias
    vectors; biases applied on PSUM eviction (per-partition ACT bias where
    the layout allows, broadcast tiles + DVE adds otherwise).
  - The S x S decay matmul is block-banded: with alpha=0.9, W's off-diagonal
    128-blocks decay by alpha^128 ~ 1.4e-6 per lag, so W @ z reduces exactly
    (to f32 resolution) to a block-diagonal matmul + NLAG lag-block matmuls
    whose matrices are shared across blocks. O(S*T*B*D) instead of
    O(S^2*B*D), with no serial carry chain.
  - Matmuls run in float32r (1 cyc/row at N>=256 vs 4 for plain f32;
    ~1.5e-4 relative rounding) accumulating in fp32 PSUM. Every f32r matmul
    input is produced by a rounding writer (DVE/ACT ops or gpsimd casting
    DMA) to satisfy the BIR verifier.
  - Software pipelined in 512-token steps: step i+1's load/LN1/transpose and
    projection matmuls are emitted between step i's mixing and FFN so the
    TensorEngine always has front-end work while LN chains run on DVE/ACT.
  - activations transposed on the PE (f32r transpose mode) since the
    contraction dim must sit on partitions for both matmul operands.
"""

import sys
from contextlib import ExitStack

for _p in ("/opt/trn_rl_repo", "/opt/pypackages"):
    if _p not in sys.path:
        sys.path.insert(0, _p)

import numpy as np

import concourse.bass as bass
import concourse.mybir as mybir
import concourse.tile as tile
from concourse import bacc
from concourse.bass_utils import run_bass_kernel_spmd
from concourse.masks import make_identity

ALPHA, EPS = 0.9, 1e-5
S, B, D, FF = 2048, 16, 512, 2048
NCORES = 8
BL = B // NCORES            # batch items per core
T = 128                     # mixing block
CB = 4                      # blocks per chunk
NBLK = S // T               # 16
NCHUNK = NBLK // CB         # 4
HC = 256                    # FFN half-chunk tokens
NFT = FF // 128             # 16 f-tiles
KD = D // 128               # 4 d-tiles
NLAG = 1                    # decay lag blocks kept (lag>=2 < 2e-12 relative)

F32 = mybir.dt.float32
F32R = mybir.dt.float32r
USE_F32R = True
MMDT = F32R if USE_F32R else F32


AF = mybir.ActivationFunctionType


def _host_consts():
    """Decay-matrix derived constants, f64 -> f32 (mirrors reference)."""
    i = np.arange(S, dtype=np.float64)
    diff = i[:, None] - i[None, :]
    with np.errstate(under="ignore"):
        W = np.where(diff >= 0, ALPHA ** (diff + 1), 0.0)
        W = W + np.diag(1.0 - W.sum(axis=1))
        W = W.astype(np.float32)
        # per-block transposed diag-blocks (lhsT of the local mixing matmul)
        blocks = [
            np.ascontiguousarray(W[c * T : (c + 1) * T, c * T : (c + 1) * T].T)
            for c in range(NBLK)
        ]
        # dedupe identical blocks (diag correction saturates after ~block 1)
        uniq, idx = [], []
        for blk in blocks:
            for j, u in enumerate(uniq):
                if np.array_equal(blk, u):
                    idx.append(j)
                    break
            else:
                idx.append(len(uniq))
                uniq.append(blk)
        wblkT = np.stack(uniq)  # [NU, T, T]
        # lag matrices: W[i0:i0+T, i0-l*T:i0-(l-1)*T] is constant across i0
        lags = []
        for l in range(1, NLAG + 1):
            L = W[l * T : (l + 1) * T, 0:T]
            for i0 in range(l * T, S, T):
                assert np.array_equal(W[i0 : i0 + T, i0 - l * T : i0 - (l - 1) * T], L)
            lags.append(np.ascontiguousarray(L.T))
        wlagT = np.stack(lags)  # [NLAG, T, T]
    return wblkT.astype(np.float32), idx, wlagT.astype(np.float32)


_WBLKT, _BLKIDX, _WLAGT = _host_consts()
NU = _WBLKT.shape[0]

_NC_CACHE = {}


def build_nc():
    key = MMDT
    if key in _NC_CACHE:
        return _NC_CACHE[key]
    nc = bacc.Bacc()

    x_d = nc.declare_dram_parameter("x", [S, BL, D], F32, isOutput=False)
    wp_d = nc.declare_dram_parameter("wp", [D, D], F32, isOutput=False)
    zb_d = nc.declare_dram_parameter("zb", [D], F32, isOutput=False)
    w1t_d = nc.declare_dram_parameter("w1t", [D, FF], F32, isOutput=False)
    hb_d = nc.declare_dram_parameter("hb", [FF], F32, isOutput=False)
    w2t_d = nc.declare_dram_parameter("w2t", [FF, D], F32, isOutput=False)
    b2_d = nc.declare_dram_parameter("b2", [D], F32, isOutput=False)
    wblk_d = nc.declare_dram_parameter("wblk", [NU, T, T], F32, isOutput=False)
    wlag_d = nc.declare_dram_parameter("wlag", [NLAG, T, T], F32, isOutput=False)
    out_d = nc.declare_dram_parameter("out", [S, BL, D], F32, isOutput=True)

    with tile.TileContext(nc) as tc, ExitStack() as ctx:
            pool = lambda name, bufs, **kw: ctx.enter_context(
                tc.tile_pool(name=name, bufs=bufs, **kw)
            )
            wgt = pool("wgt", 1)
            stage = pool("stage", 1)
            xin = pool("xin", 9)
            lnp = pool("ln", 6)
            yppp = pool("ypp", 2)
            xtp = pool("xt", 6)
            y2tp = pool("y2t", 2)
            zp = pool("z", 10)
            x2p = pool("x2", 5)
            hp = pool("h", 2)
            outp = pool("outp", 3)
            psmm = pool("psmm", 5, space="PSUM")
            pstr = pool("pstr", 3, space="PSUM")
            # ---------------- one-time setup ----------------

            xpre = {}

            def preload_x(i):
                b, c = steps[i]
                tiles = []
                for t in range(CB):
                    s0 = (c * CB + t) * T
                    xt = xin.tile([128, D], F32, tag="x")
                    nc.sync.dma_start(xt[:], x_d.ap()[s0 : s0 + T, b, :])
                    tiles.append(xt)
                xpre[i] = tiles

            steps = [(b, c) for b in range(BL) for c in range(NCHUNK)]
            preload_x(0)
            ident_f = stage.tile([128, 128], F32, tag="ident_f")
            make_identity(nc, ident_f[:])
            ident = wgt.tile([128, 128], MMDT, tag="ident")
            nc.vector.tensor_copy(ident[:], ident_f[:])
            eps_t = wgt.tile([128, 1], F32, tag="eps")
            nc.vector.memset(eps_t[:], EPS)
            neg1_t = wgt.tile([128, 1], F32, tag="neg1")
            nc.vector.memset(neg1_t[:], -1.0)
            zb_bc = wgt.tile([128, D], F32, tag="zb")
            nc.sync.dma_start(
                zb_bc[:],
                bass.AP(tensor=zb_d, offset=0, ap=[[0, 128], [1, D]]),
            )
            b2_bc = wgt.tile([128, D], F32, tag="b2")
            nc.sync.dma_start(
                b2_bc[:],
                bass.AP(tensor=b2_d, offset=0, ap=[[0, 128], [1, D]]),
            )
            hb_sb = wgt.tile([128, NFT], F32, tag="hb")
            nc.sync.dma_start(
                hb_sb[:],
                bass.AP(tensor=hb_d, offset=0, ap=[[1, 128], [128, NFT]]),
            )

            def load_round(dram_ap, shape, tag):
                """Casting DMA (gpsimd SWDGE) f32 DRAM -> resident f32r tile."""
                rt = wgt.tile(shape, MMDT, tag=tag)
                nc.gpsimd.dma_start(rt[:], dram_ap)
                return rt

            wp_r = load_round(
                wp_d.ap().rearrange("(kd p) e -> p kd e", p=128), [128, KD, D], "wp"
            )
            wblk_r = load_round(
                wblk_d.ap().rearrange("b j r -> j b r"), [128, NU, T], "wblk"
            )
            wlag_r = load_round(
                wlag_d.ap().rearrange("b j r -> j b r"), [128, NLAG, T], "wlag"
            )

            # ---------------- helpers ----------------
            def layer_norm_stats(xt):
                """-> (mean, rstd) [128,1] tiles."""
                st = lnp.tile([128, 6], F32, tag="bnst")
                nc.vector.bn_stats(st[:], xt)
                mv = lnp.tile([128, 2], F32, tag="bnmv")
                nc.vector.bn_aggr(mv[:], st[:])
                rstd = lnp.tile([128, 1], F32, tag="rstd")
                nc.scalar.activation(
                    rstd[:], mv[:, 1:2], AF.Sqrt, bias=eps_t[:], scale=1.0
                )
                nc.vector.reciprocal(rstd[:], rstd[:])
                return mv, rstd

            def normalize_transpose(xt, tag, dest, dest_off, use_dve=False, stats=None):
                """LN(xt) -> transposed [d, s] written into dest[:, :, off:off+128]."""
                mv, rstd = stats if stats is not None else layer_norm_stats(xt)
                ypp = yppp.tile([128, D], MMDT, tag=tag)
                if use_dve:
                    nc.vector.tensor_scalar(
                        out=ypp[:],
                        in0=xt,
                        scalar1=mv[:, 0:1],
                        scalar2=rstd[:],
                        op0=mybir.AluOpType.subtract,
                        op1=mybir.AluOpType.mult,
                    )
                else:
                    nbias = lnp.tile([128, 1], F32, tag="nbias")
                    nc.vector.tensor_scalar(
                        out=nbias[:],
                        in0=rstd[:],
                        scalar1=mv[:, 0:1],
                        scalar2=neg1_t[:],
                        op0=mybir.AluOpType.mult,
                        op1=mybir.AluOpType.mult,
                    )
                    nc.scalar.activation(
                        ypp[:], xt, AF.Identity, bias=nbias[:], scale=rstd[:]
                    )
                pt = pstr.tile([128, D], MMDT, tag="tr")
                for kd in range(KD):
                    nc.tensor.transpose(
                        pt[:, kd * 128 : (kd + 1) * 128],
                        ypp[:, kd * 128 : (kd + 1) * 128],
                        ident[:],
                    )
                nc.scalar.activation(
                    dest[:, :, dest_off : dest_off + 128],
                    pt[:].rearrange("p (a b) -> p a b", b=128),
                    AF.Copy,
                )

            # ---------------- main pipeline ----------------
            # Software-pipelined across steps (a step = 4 blocks = 512 tokens):
            #   iter i emits: mixD(i) | stageA(i+1) | projMM(i+1) | LN2+transp(i)
            #                 | z-evict(i+1) | FFN1(i) | FFN2(i)
            # so PE always has front-end work of step i+1 while step i's
            # LN chains run on DVE/ACT/GPSIMD.
            zall = {b: [] for b in range(BL)}
            a_out, b_out = {}, {}

            def stage_a(i):
                b, c = steps[i]
                if i not in xpre:
                    preload_x(i)
                xts, xT = xpre.pop(i), []
                stats = [layer_norm_stats(xts[t][:]) for t in range(CB)]
                for t in range(CB):
                    xTt = xtp.tile([128, KD, 128], MMDT, tag="xT")
                    normalize_transpose(xts[t][:], "ypp", xTt, 0, stats=stats[t])
                    xT.append(xTt)
                a_out[i] = (xts, xT)

            def stage_b_mm(i):
                _, xT = a_out[i]
                pzs = []
                for t in range(CB):
                    pz = psmm.tile([128, D], F32, tag="mm")
                    for kd in range(KD):
                        nc.tensor.matmul(
                            pz[:],
                            xT[t][:, kd, :],
                            wp_r[:, kd, :],
                            start=(kd == 0),
                            stop=(kd == KD - 1),
                        )
                    pzs.append(pz)
                b_out[i] = pzs

            def stage_b_evict(i):
                b, c = steps[i]
                for t in range(CB):
                    zt = zp.tile([128, D], MMDT, tag="z")
                    nc.vector.tensor_add(zt[:], b_out[i][t][:], zb_bc[:])
                    zall[b].append(zt)
                del b_out[i]

            stage_a(0)
            stage_b_mm(0)
            stage_b_evict(0)
            # big FFN weights: allocated now, DMA'd in chunks interleaved with
            # the early pipeline so x loads and the first FFN aren't blocked
            # behind 16 MB of weight traffic.
            w1t_r = wgt.tile([128, KD, FF], MMDT, tag="w1t")
            w2t_r = wgt.tile([128, NFT, D], MMDT, tag="w2t")
            w1t_ap = w1t_d.ap().rearrange("(kd p) f -> p kd f", p=128)
            w2t_ap = w2t_d.ap().rearrange("(kf p) d -> p kf d", p=128)
            wload = [
                lambda kd=kd: nc.gpsimd.dma_start(w1t_r[:, kd, :], w1t_ap[:, kd, :])
                for kd in range(KD)
            ] + [
                lambda f4=f4: nc.gpsimd.dma_start(
                    w2t_r[:, 4 * f4 : 4 * f4 + 4, :], w2t_ap[:, 4 * f4 : 4 * f4 + 4, :]
                )
                for f4 in range(4)
            ]
            wload.reverse()  # pop() from the front
            if wload:
                wload.pop()()  # w1t kd=0 immediately
            for i, (b, c) in enumerate(steps):
                xts, _ = a_out.pop(i)
                x2ts, pms = [], []
                # --- mixing (banded); evicts deferred to DVE below ---
                for t in range(CB):
                    blk = c * CB + t
                    nmix = 1 + min(blk, NLAG)
                    pm = psmm.tile([128, D], F32, tag="mm")
                    nc.tensor.matmul(
                        pm[:],
                        wblk_r[:, _BLKIDX[blk], :],
                        zall[b][blk][:],
                        start=True,
                        stop=(nmix == 1),
                    )
                    for l in range(1, nmix):
                        nc.tensor.matmul(
                            pm[:],
                            wlag_r[:, l - 1, :],
                            zall[b][blk - l][:],
                            start=False,
                            stop=(l == nmix - 1),
                        )
                    pms.append(pm)
                # --- prefetch next step stage A (its LN chain runs while the
                # mixing evicts and LN2 chain of this step proceed) ---
                if i + 1 < len(steps):
                    stage_a(i + 1)
                if wload:
                    wload.pop()()
                last = i + 1 == len(steps)
                stats2 = []
                for t in range(CB):
                    x2t = x2p.tile([128, D], F32, tag="x2")
                    nc.vector.tensor_add(x2t[:], pms[t][:], xts[t][:])
                    x2ts.append(x2t)
                    if last:
                        # tail: no next step to hide behind -- start LN2 stats
                        # right after each mixing eviction
                        stats2.append(layer_norm_stats(x2t[:]))
                if i + 1 < len(steps):
                    stage_b_mm(i + 1)
                if wload:
                    wload.pop()()
                # --- LN2 + transpose (per half-chunk dest) ---
                y2T = []
                for hh in range(2):
                    y2Th = y2tp.tile([128, KD, HC], MMDT, tag="y2T")
                    y2T.append(y2Th)
                if not stats2:
                    stats2 = [layer_norm_stats(x2ts[t][:]) for t in range(CB)]
                for t in range(CB):
                    normalize_transpose(
                        x2ts[t][:], "y2pp", y2T[t // 2], (t % 2) * 128,
                        use_dve=True, stats=stats2[t],
                    )
                if wload:
                    wload.pop()()
                if i + 1 < len(steps):
                    stage_b_evict(i + 1)
                # --- FFN1 + FFN2 interleaved per half-chunk ---
                for hh in range(2):
                    ht = hp.tile([128, NFT, HC], MMDT, tag="h")
                    for ft in range(NFT):
                        ph = psmm.tile([128, HC], F32, tag="mm")
                        for kd in range(KD):
                            nc.tensor.matmul(
                                ph[:],
                                w1t_r[:, kd, ft * 128 : (ft + 1) * 128],
                                y2T[hh][:, kd, :],
                                start=(kd == 0),
                                stop=(kd == KD - 1),
                            )
                        nc.scalar.activation(
                            ht[:, ft, :],
                            ph[:],
                            AF.Relu,
                            bias=hb_sb[:, ft : ft + 1],
                            scale=1.0,
                        )
                    for tt in range(2):
                        t = 2 * hh + tt
                        s0 = (c * CB + t) * T
                        po = psmm.tile([128, D], F32, tag="mm")
                        for ft in range(NFT):
                            if wload and ft % 4 == 0:
                                wload.pop()()
                            nc.tensor.matmul(
                                po[:],
                                ht[:, ft, tt * 128 : tt * 128 + 128],
                                w2t_r[:, ft, :],
                                start=(ft == 0),
                                stop=(ft == NFT - 1),
                            )
                        ot = outp.tile([128, D], F32, tag="o")
                        nc.vector.tensor_add(ot[:], po[:], b2_bc[:])
                        nc.vector.tensor_add(ot[:], ot[:], x2ts[t][:])
                        nc.sync.dma_start(out_d.ap()[s0 : s0 + T, b, :], ot[:])

    nc.compile()
    _NC_CACHE[key] = nc
    return nc


def _prep_inputs(x, w_lin, b_lin, w1, b1, w2, b2, g1, beta1, g2, beta2):
    f32 = np.float32
    wp = np.ascontiguousarray(w_lin.T * g1[:, None]).astype(f32)
    zb = (w_lin.astype(np.float64) @ beta1.astype(np.float64) + b_lin).astype(f32)
    w1t = np.ascontiguousarray(w1.T * g2[:, None]).astype(f32)
    hb = (w1.astype(np.float64) @ beta2.astype(np.float64) + b1).astype(f32)
    w2t = np.ascontiguousarray(w2.T).astype(f32)
    shared = {
        "wp": wp,
        "zb": zb,
        "w1t": w1t,
        "hb": hb,
        "w2t": w2t,
        "b2": b2.astype(f32),
        "wblk": _WBLKT,
        "wlag": _WLAGT,
    }
    in_maps = []
    for cc in range(NCORES):
        m = dict(shared)
        m["x"] = np.ascontiguousarray(x[:, cc * BL : (cc + 1) * BL, :]).astype(f32)
        in_maps.append(m)
    return in_maps


def kernel(**inputs):
    nc = build_nc()
    in_maps = _prep_inputs(**inputs)
    res = run_bass_kernel_spmd(nc, in_maps, list(range(NCORES)))
    out = np.concatenate([r["out"] for r in res.results], axis=1)
    return out.astype(np.float32)


if __name__ == "__main__":
    rng = np.random.default_rng(0)
    demo = {
        "x": rng.standard_normal((S, B, D)).astype(np.float32),
        "w_lin": rng.standard_normal((D, D)).astype(np.float32) * D**-0.5,
        "b_lin": rng.standard_normal((D,)).astype(np.float32) * 0.01,
        "w1": rng.standard_normal((FF, D)).astype(np.float32) * D**-0.5,
        "b1": rng.standard_normal((FF,)).astype(np.float32) * 0.01,
        "w2": rng.standard_normal((D, FF)).astype(np.float32) * FF**-0.5,
        "b2": rng.standard_normal((D,)).astype(np.float32) * 0.01,
        "g1": np.ones(D, np.float32),
        "beta1": np.zeros(D, np.float32),
        "g2": np.ones(D, np.float32),
        "beta2": np.zeros(D, np.float32),
    }
    out = kernel(**demo)
    print("ok", out.shape, out.dtype)



# revision 7
# speedup vs baseline: 1.3979x; 1.3979x over previous
"""Trainium2 Bass kernel for nn_ExpSelfAttention (dense transformer block).

Math (per batch item b, all f32 data):
    y  = LN(x; g1, beta1);  z = y @ w_lin.T + b_lin
    attn = W @ z            (W = causal exp-decay matrix, alpha=0.9)
    x2 = x + attn
    y2 = LN(x2; g2, beta2); h = relu(y2 @ w1.T + b1)
    out = x2 + h @ w2.T + b2

Sharding: data parallel over batch (16 / 8 cores = 2 per core); weights and
the (input-independent) decay-matrix blocks replicated. No collectives.

Precision plan (rel-err budget 2e-2, this lands ~5e-3):
  - FFN matmuls in fp8-e4m3 with MatmulPerfMode.DoubleRow (packs two
    contraction rows per PE cell: 0.5 cyc/output-row and K=256 per
    instruction -> 4x the f32r FLOP rate). y2/h/w1/w2 quantized to fp8.
  - Projection in bf16 (feeds the decay mixing whose output dominates the
    result -- fp8 there would blow the error budget); mixing in f32r.
  - x2 residual held in bf16; final output assembled in f32.

Engine balance (per-batch-item busy, approx): PE 55us (proj 14, mix 7,
FFN 27, transposes 7), DVE 53us (bn_stats/aggr, z+x2 PSUM evict-adds,
xT evict copies, 3/16 of the relus), Act 50us (relu 13/16, y2T evicts,
final out copies, sqrt), Pool/gpsimd 44us (both LN normalizes, x2+b2).
b2 and x2 are folded into the FFN2 PSUM accumulation via an identity-
weight matmul so the output eviction is a pure Act copy.

All big weights are pre-cast on the host and passed as fp8/bf16 DRAM
parameters (halves weight DMA traffic; no on-chip cast pass).
"""

import sys
from contextlib import ExitStack

for _p in ("/opt/trn_rl_repo", "/opt/pypackages"):
    if _p not in sys.path:
        sys.path.insert(0, _p)

import numpy as np
import ml_dtypes

import concourse.bass as bass
import concourse.mybir as mybir
import concourse.tile as tile
from concourse import bacc
from concourse.bass_utils import run_bass_kernel_spmd
from concourse.masks import make_identity

ALPHA, EPS = 0.9, 1e-5
S, B, D, FF = 2048, 16, 512, 2048
NCORES = 8
BL = B // NCORES            # batch items per core
T = 128                     # token tile
CB = 4                      # token tiles per chunk
NBLK = S // T               # 16
NCHUNK = NBLK // CB         # 4
NFT = FF // 128             # 16 f-tiles
KD = D // 128               # 4 d-tiles
NLAG = 1                    # decay lag blocks kept (lag>=2 < 2e-12 relative)
N_RELU_DVE = 3              # of the 16 relus per chunk, run this many on DVE

F32 = mybir.dt.float32
F32R = mybir.dt.float32r
BF16 = mybir.dt.bfloat16
F8 = mybir.dt.float8e4
AF = mybir.ActivationFunctionType
ALU = mybir.AluOpType
DR = mybir.MatmulPerfMode.DoubleRow

NP_F8 = ml_dtypes.float8_e4m3
NP_BF16 = ml_dtypes.bfloat16


def _host_consts():
    """Decay-matrix derived constants, f64 -> f32 (mirrors reference)."""
    i = np.arange(S, dtype=np.float64)
    diff = i[:, None] - i[None, :]
    with np.errstate(under="ignore"):
        W = np.where(diff >= 0, ALPHA ** (diff + 1), 0.0)
        W = W + np.diag(1.0 - W.sum(axis=1))
        W = W.astype(np.float32)
        blocks = [
            np.ascontiguousarray(W[c * T : (c + 1) * T, c * T : (c + 1) * T].T)
            for c in range(NBLK)
        ]
        uniq, idx = [], []
        for blk in blocks:
            for j, u in enumerate(uniq):
                if np.array_equal(blk, u):
                    idx.append(j)
                    break
            else:
                idx.append(len(uniq))
                uniq.append(blk)
        wblkT = np.stack(uniq)  # [NU, T, T]
        lags = []
        for l in range(1, NLAG + 1):
            L = W[l * T : (l + 1) * T, 0:T]
            for i0 in range(l * T, S, T):
                assert np.array_equal(W[i0 : i0 + T, i0 - l * T : i0 - (l - 1) * T], L)
            lags.append(np.ascontiguousarray(L.T))
        wlagT = np.stack(lags)  # [NLAG, T, T]
    return wblkT.astype(np.float32), idx, wlagT.astype(np.float32)


_WBLKT, _BLKIDX, _WLAGT = _host_consts()
NU = _WBLKT.shape[0]

_NC_CACHE = {}


def build_nc():
    key = 0
    if key in _NC_CACHE:
        return _NC_CACHE[key]
    nc = bacc.Bacc()

    x_d = nc.declare_dram_parameter("x", [S, BL, D], F32, isOutput=False)
    wp_d = nc.declare_dram_parameter("wp", [D, D], BF16, isOutput=False)
    zb_d = nc.declare_dram_parameter("zb", [D], F32, isOutput=False)
    w1t_d = nc.declare_dram_parameter("w1t", [D, FF], F8, isOutput=False)
    hb_d = nc.declare_dram_parameter("hb", [FF], F32, isOutput=False)
    w2t_d = nc.declare_dram_parameter("w2t", [FF, D], F8, isOutput=False)
    b2_d = nc.declare_dram_parameter("b2", [D], F32, isOutput=False)
    wblk_d = nc.declare_dram_parameter("wblk", [NU, T, T], F32, isOutput=False)
    wlag_d = nc.declare_dram_parameter("wlag", [NLAG, T, T], F32, isOutput=False)
    out_d = nc.declare_dram_parameter("out", [S, BL, D], F32, isOutput=True)

    with tile.TileContext(nc) as tc, ExitStack() as ctx:
        pool = lambda name, bufs, **kw: ctx.enter_context(
            tc.tile_pool(name=name, bufs=bufs, **kw)
        )
        wgt = pool("wgt", 1)
        stage = pool("stage", 1)
        xin = pool("xin", 9)
        lnp = pool("ln", 4)
        yppp = pool("ypp", 3)
        xtp = pool("xt", 6)
        y2tp = pool("y2t", 2)
        zp = pool("z", 10)
        x2p = pool("x2", 5)
        hp = pool("h", 2)
        outp = pool("outp", 3)
        # single full-bank PSUM rotation; transposes write through bitcast
        # views so bf16/fp8 tiles share the same bank pool
        psmm = pool("psmm", 8, space="PSUM")

        # ---------------- one-time setup ----------------
        xpre = {}

        def preload_x(i):
            b, c = steps[i]
            tiles = []
            for t in range(CB):
                s0 = (c * CB + t) * T
                xt = xin.tile([128, D], F32, tag="x")
                nc.sync.dma_start(xt[:], x_d.ap()[s0 : s0 + T, b, :])
                tiles.append(xt)
            xpre[i] = tiles

        steps = [(b, c) for b in range(BL) for c in range(NCHUNK)]
        preload_x(0)
        ident_f = stage.tile([128, 128], F32, tag="ident_f")
        make_identity(nc, ident_f[:])
        identB = wgt.tile([128, 128], BF16, tag="identB")
        nc.vector.tensor_copy(identB[:], ident_f[:])
        ident8 = wgt.tile([128, 128], F8, tag="ident8")
        nc.vector.tensor_copy(ident8[:], ident_f[:])
        eps_t = wgt.tile([128, 1], F32, tag="eps")
        nc.vector.memset(eps_t[:], EPS)
        zb_bc = wgt.tile([128, D], F32, tag="zb")
        nc.sync.dma_start(
            zb_bc[:], bass.AP(tensor=zb_d, offset=0, ap=[[0, 128], [1, D]])
        )
        b2_bc = wgt.tile([128, D], F32, tag="b2")
        nc.sync.dma_start(
            b2_bc[:], bass.AP(tensor=b2_d, offset=0, ap=[[0, 128], [1, D]])
        )
        b2b = wgt.tile([128, D], BF16, tag="b2b")
        nc.vector.tensor_copy(b2b[:], b2_bc[:])
        hb_sb = wgt.tile([128, NFT], F32, tag="hb")
        nc.sync.dma_start(
            hb_sb[:], bass.AP(tensor=hb_d, offset=0, ap=[[1, 128], [128, NFT]])
        )

        # mixing matrices: f32 DRAM -> resident f32r via casting DMA (SWDGE)
        wblk_r = wgt.tile([128, NU, T], F32R, tag="wblk")
        nc.gpsimd.dma_start(wblk_r[:], wblk_d.ap().rearrange("b j r -> j b r"))
        wlag_r = wgt.tile([128, NLAG, T], F32R, tag="wlag")
        nc.gpsimd.dma_start(wlag_r[:], wlag_d.ap().rearrange("b j r -> j b r"))

        # projection weight: bf16 straight from DRAM
        wp_r = wgt.tile([128, KD, D], BF16, tag="wp")
        nc.sync.dma_start(wp_r[:], wp_d.ap().rearrange("(kd p) e -> p kd e", p=128))

        # ---------------- helpers ----------------
        def ln_stats(xts, tag):
            """4 tiles -> (mv4 [128,4,2], r4 [128,4]) batched sqrt+recip."""
            mv4 = lnp.tile([128, CB, 2], F32, tag=f"mv4{tag}")
            for t in range(CB):
                st = lnp.tile([128, 6], F32, tag=f"bnst{tag}")
                nc.vector.bn_stats(st[:], xts[t][:])
                nc.vector.bn_aggr(mv4[:, t, :], st[:])
            r4 = lnp.tile([128, CB], F32, tag=f"r4{tag}")
            nc.scalar.activation(
                r4[:], mv4[:, :, 1], AF.Sqrt, bias=eps_t[:], scale=1.0
            )
            nc.vector.reciprocal(r4[:], r4[:])
            return mv4, r4

        # ---------------- main pipeline ----------------
        zall = {b: [] for b in range(BL)}
        a_out, b_out = {}, {}

        def stage_ln1(i):
            """LN1 + transpose: -> xT tiles (bf16, [d, kd, tok])."""
            b, c = steps[i]
            if i not in xpre:
                preload_x(i)
            xts = xpre.pop(i)
            mv4, r4 = ln_stats(xts, "a")
            xT = []
            for t in range(CB):
                ypp = yppp.tile([128, D], BF16, tag="ypp")
                nc.gpsimd.tensor_scalar(
                    out=ypp[:],
                    in0=xts[t][:],
                    scalar1=mv4[:, t, 0:1],
                    scalar2=r4[:, t : t + 1],
                    op0=ALU.subtract,
                    op1=ALU.mult,
                )
                ptb = psmm.tile([128, D], F32, tag="mm")
                pt = ptb[:].bitcast(BF16)  # [128, 1024] view, use cols 0..511
                for kd in range(KD):
                    nc.tensor.transpose(
                        pt[:, kd * 128 : (kd + 1) * 128],
                        ypp[:, kd * 128 : (kd + 1) * 128],
                        identB[:],
                    )
                xTt = xtp.tile([128, KD, 128], BF16, tag="xT")
                nc.vector.tensor_copy(
                    xTt[:], pt[:, 0:D].rearrange("p (a b) -> p a b", b=128)
                )
                xT.append(xTt)
            a_out[i] = (xts, xT)

        def stage_proj(i):
            _, xT = a_out[i]
            pzs = []
            for t in range(CB):
                pz = psmm.tile([128, D], F32, tag="mm")
                for kd in range(KD):
                    nc.tensor.matmul(
                        pz[:],
                        xT[t][:, kd, :],
                        wp_r[:, kd, :],
                        start=(kd == 0),
                        stop=(kd == KD - 1),
                    )
                pzs.append(pz)
            b_out[i] = pzs

        def stage_zev(i):
            b, c = steps[i]
            for t in range(CB):
                zt = zp.tile([128, D], F32R, tag="z")
                nc.vector.tensor_add(zt[:], b_out[i][t][:], zb_bc[:])
                zall[b].append(zt)
            del b_out[i]

        stage_ln1(0)
        stage_proj(0)
        stage_zev(0)

        # big fp8 weights: DMA'd in chunks interleaved with the early
        # pipeline so x loads aren't blocked behind the weight traffic.
        w1t_r = wgt.tile([128, KD, FF], F8, tag="w1t")
        w2t_r = wgt.tile([128, NFT, D], F8, tag="w2t")
        w1t_ap = w1t_d.ap().rearrange("(kd p) f -> p kd f", p=128)
        w2t_ap = w2t_d.ap().rearrange("(kf p) d -> p kf d", p=128)
        wload = [
            lambda kd2=kd2: nc.sync.dma_start(
                w1t_r[:, 2 * kd2 : 2 * kd2 + 2, :], w1t_ap[:, 2 * kd2 : 2 * kd2 + 2, :]
            )
            for kd2 in range(KD // 2)
        ] + [
            lambda f8=f8: nc.sync.dma_start(
                w2t_r[:, 8 * f8 : 8 * f8 + 8, :], w2t_ap[:, 8 * f8 : 8 * f8 + 8, :]
            )
            for f8 in range(2)
        ]
        wload.reverse()  # pop() from the front
        wload.pop()()

        for i, (b, c) in enumerate(steps):
            xts, _ = a_out.pop(i)
            # --- mixing (banded decay matmul) ---
            pms = []
            for t in range(CB):
                blk = c * CB + t
                nmix = 1 + min(blk, NLAG)
                pm = psmm.tile([128, D], F32, tag="mm")
                nc.tensor.matmul(
                    pm[:],
                    wblk_r[:, _BLKIDX[blk], :],
                    zall[b][blk][:],
                    start=True,
                    stop=(nmix == 1),
                )
                for l in range(1, nmix):
                    nc.tensor.matmul(
                        pm[:],
                        wlag_r[:, l - 1, :],
                        zall[b][blk - l][:],
                        start=False,
                        stop=(l == nmix - 1),
                    )
                pms.append(pm)
            if i + 1 < len(steps):
                preload_x(i + 1)
            # --- x2 = x + attn (DVE, bf16) then x2+b2 (Pool) ---
            x2ts, x2pbs = [], []
            for t in range(CB):
                x2t = x2p.tile([128, D], BF16, tag="x2")
                nc.vector.tensor_add(x2t[:], pms[t][:], xts[t][:])
                x2ts.append(x2t)
            for t in range(CB):
                x2pb = x2p.tile([128, D], BF16, tag="x2pb")
                nc.gpsimd.tensor_add(x2pb[:], x2ts[t][:], b2b[:])
                x2pbs.append(x2pb)
            # --- next step's LN1 + projection (keeps PE fed) ---
            if i + 1 < len(steps):
                stage_ln1(i + 1)
            if wload:
                wload.pop()()
            # --- LN2 -> y2T (fp8, [d, kd, tok(chunk)]) ---
            mv4b, r4b = ln_stats(x2ts, "b")
            y2T = y2tp.tile([128, KD, CB * 128], F8, tag="y2T")
            for t in range(CB):
                y2pp = yppp.tile([128, D], F8, tag="y2pp")
                nc.gpsimd.tensor_scalar(
                    out=y2pp[:],
                    in0=x2ts[t][:],
                    scalar1=mv4b[:, t, 0:1],
                    scalar2=r4b[:, t : t + 1],
                    op0=ALU.subtract,
                    op1=ALU.mult,
                )
                pt2b = psmm.tile([128, D], F32, tag="mm")
                pt2 = pt2b[:].bitcast(F8)  # [128, 2048] view, use cols 0..511
                for kd in range(KD):
                    nc.tensor.transpose(
                        pt2[:, kd * 128 : (kd + 1) * 128],
                        y2pp[:, kd * 128 : (kd + 1) * 128],
                        ident8[:],
                    )
                nc.scalar.activation(
                    y2T[:, :, t * 128 : (t + 1) * 128],
                    pt2[:, 0:D].rearrange("p (a b) -> p a b", b=128),
                    AF.Copy,
                )
            if i + 1 < len(steps):
                stage_proj(i + 1)
            if wload:
                wload.pop()()
            if i + 1 < len(steps):
                stage_zev(i + 1)
            if wload:
                wload.pop()()
            # --- FFN1: fp8 DoubleRow + relu (split Act/DVE) ---
            ht = hp.tile([128, NFT, CB * 128], F8, tag="h")
            for ft in range(NFT):
                ph = psmm.tile([128, CB * 128], F32, tag="mm")
                for j in range(KD // 2):
                    nc.tensor.matmul(
                        ph[:],
                        w1t_r[:, 2 * j : 2 * j + 2, ft * 128 : (ft + 1) * 128],
                        y2T[:, 2 * j : 2 * j + 2, :],
                        start=(j == 0),
                        stop=(j == KD // 2 - 1),
                        perf_mode=DR,
                    )
                if ft in (5, 10, 15)[:N_RELU_DVE]:
                    nc.vector.tensor_scalar(
                        out=ht[:, ft, :],
                        in0=ph[:],
                        scalar1=hb_sb[:, ft : ft + 1],
                        scalar2=0.0,
                        op0=ALU.add,
                        op1=ALU.max,
                    )
                else:
                    nc.scalar.activation(
                        ht[:, ft, :],
                        ph[:],
                        AF.Relu,
                        bias=hb_sb[:, ft : ft + 1],
                        scale=1.0,
                    )
            # --- FFN2: fp8 DoubleRow + (x2+b2) via identity matmul ---
            for t in range(CB):
                s0 = (c * CB + t) * T
                po = psmm.tile([128, D], F32, tag="mm")
                for j in range(NFT // 2):
                    nc.tensor.matmul(
                        po[:],
                        ht[:, 2 * j : 2 * j + 2, t * 128 : (t + 1) * 128],
                        w2t_r[:, 2 * j : 2 * j + 2, :],
                        start=(j == 0),
                        stop=False,
                        perf_mode=DR,
                    )
                nc.tensor.matmul(
                    po[:], identB[:], x2pbs[t][:], start=False, stop=True
                )
                ot = outp.tile([128, D], F32, tag="o")
                nc.scalar.activation(ot[:], po[:], AF.Copy)
                nc.sync.dma_start(out_d.ap()[s0 : s0 + T, b, :], ot[:])

    nc.compile()
    _NC_CACHE[key] = nc
    return nc


def _prep_inputs(x, w_lin, b_lin, w1, b1, w2, b2, g1, beta1, g2, beta2):
    f32 = np.float32
    wp = np.ascontiguousarray(w_lin.T * g1[:, None]).astype(NP_BF16)
    zb = (w_lin.astype(np.float64) @ beta1.astype(np.float64) + b_lin).astype(f32)
    w1t = np.ascontiguousarray(w1.T * g2[:, None]).astype(NP_F8)
    hb = (w1.astype(np.float64) @ beta2.astype(np.float64) + b1).astype(f32)
    w2t = np.ascontiguousarray(w2.T).astype(NP_F8)
    shared = {
        "wp": wp,
        "zb": zb,
        "w1t": w1t,
        "hb": hb,
        "w2t": w2t,
        "b2": b2.astype(f32),
        "wblk": _WBLKT,
        "wlag": _WLAGT,
    }
    in_maps = []
    for cc in range(NCORES):
        m = dict(shared)
        m["x"] = np.ascontiguousarray(x[:, cc * BL : (cc + 1) * BL, :]).astype(f32)
        in_maps.append(m)
    return in_maps


def kernel(**inputs):
    nc = build_nc()
    in_maps = _prep_inputs(**inputs)
    res = run_bass_kernel_spmd(nc, in_maps, list(range(NCORES)))
    out = np.concatenate([r["out"] for r in res.results], axis=1)
    return out.astype(np.float32)


if __name__ == "__main__":
    rng = np.random.default_rng(0)
    demo = {
        "x": rng.standard_normal((S, B, D)).astype(np.float32),
        "w_lin": rng.standard_normal((D, D)).astype(np.float32) * D**-0.5,
        "b_lin": rng.standard_normal((D,)).astype(np.float32) * 0.01,
        "w1": rng.standard_normal((FF, D)).astype(np.float32) * D**-0.5,
        "b1": rng.standard_normal((FF,)).astype(np.float32) * 0.01,
        "w2": rng.standard_normal((D, FF)).astype(np.float32) * FF**-0.5,
        "b2": rng.standard_normal((D,)).astype(np.float32) * 0.01,
        "g1": np.ones(D, np.float32),
        "beta1": np.zeros(D, np.float32),
        "g2": np.ones(D, np.float32),
        "beta2": np.zeros(D, np.float32),
    }
    out = kernel(**demo)
    print("ok", out.shape, out.dtype)


# revision 9
# speedup vs baseline: 1.7150x; 1.2269x over previous
"""Trainium2 Bass kernel for nn_ExpSelfAttention (dense transformer block).

Math (per batch item b, all f32 data):
    y  = LN(x; g1, beta1);  z = y @ w_lin.T + b_lin
    attn = W @ z            (W = causal exp-decay matrix, alpha=0.9)
    x2 = x + attn
    y2 = LN(x2; g2, beta2); h = relu(y2 @ w1.T + b1)
    out = x2 + h @ w2.T + b2

Sharding: data parallel over batch (16 / 8 cores = 2 per core); weights and
the (input-independent) decay-matrix blocks replicated. No collectives.

Precision plan (rel-err budget 2e-2, this lands ~5e-3):
  - FFN matmuls in fp8-e4m3 with MatmulPerfMode.DoubleRow (packs two
    contraction rows per PE cell: 0.5 cyc/output-row and K=256 per
    instruction -> 4x the f32r FLOP rate). y2/h/w1/w2 quantized to fp8.
  - Projection in bf16 (feeds the decay mixing whose output dominates the
    result -- fp8 there would blow the error budget); mixing in f32r.
  - x2 residual held in bf16; final output assembled in f32.

Engine balance (per-batch-item busy, approx): PE 55us (proj 14, mix 7,
FFN 27, transposes 7), DVE 53us (bn_stats/aggr, z+x2 PSUM evict-adds,
xT evict copies, 3/16 of the relus), Act 50us (relu 13/16, y2T evicts,
final out copies, sqrt), Pool/gpsimd 44us (both LN normalizes, x2+b2).
b2 and x2 are folded into the FFN2 PSUM accumulation via an identity-
weight matmul so the output eviction is a pure Act copy.

All big weights are pre-cast on the host and passed as fp8/bf16 DRAM
parameters (halves weight DMA traffic; no on-chip cast pass).
"""

import sys
from contextlib import ExitStack

for _p in ("/opt/trn_rl_repo", "/opt/pypackages"):
    if _p not in sys.path:
        sys.path.insert(0, _p)

import numpy as np
import ml_dtypes

import concourse.bass as bass
import concourse.mybir as mybir
import concourse.tile as tile
from concourse import bacc
from concourse.bass_utils import run_bass_kernel_spmd
from concourse.masks import make_identity

ALPHA, EPS = 0.9, 1e-5
S, B, D, FF = 2048, 16, 512, 2048
NCORES = 8
BL = B // NCORES            # batch items per core
T = 128                     # token tile
CB = 4                      # token tiles per chunk
NBLK = S // T               # 16
NCHUNK = NBLK // CB         # 4
NFT = FF // 128             # 16 f-tiles
KD = D // 128               # 4 d-tiles
NLAG = 1                    # decay lag blocks kept (lag>=2 < 2e-12 relative)
N_RELU_DVE = 3              # of the 16 relus per chunk, run this many on DVE

F32 = mybir.dt.float32
F32R = mybir.dt.float32r
BF16 = mybir.dt.bfloat16
F8 = mybir.dt.float8e4
AF = mybir.ActivationFunctionType
ALU = mybir.AluOpType
DR = mybir.MatmulPerfMode.DoubleRow

NP_F8 = ml_dtypes.float8_e4m3
NP_BF16 = ml_dtypes.bfloat16


def _host_consts():
    """Decay-matrix derived constants, f64 -> f32 (mirrors reference)."""
    i = np.arange(S, dtype=np.float64)
    diff = i[:, None] - i[None, :]
    with np.errstate(under="ignore"):
        W = np.where(diff >= 0, ALPHA ** (diff + 1), 0.0)
        W = W + np.diag(1.0 - W.sum(axis=1))
        W = W.astype(np.float32)
        blocks = [
            np.ascontiguousarray(W[c * T : (c + 1) * T, c * T : (c + 1) * T].T)
            for c in range(NBLK)
        ]
        uniq, idx = [], []
        for blk in blocks:
            for j, u in enumerate(uniq):
                if np.array_equal(blk, u):
                    idx.append(j)
                    break
            else:
                idx.append(len(uniq))
                uniq.append(blk)
        wblkT = np.stack(uniq)  # [NU, T, T]
        lags = []
        for l in range(1, NLAG + 1):
            L = W[l * T : (l + 1) * T, 0:T]
            for i0 in range(l * T, S, T):
                assert np.array_equal(W[i0 : i0 + T, i0 - l * T : i0 - (l - 1) * T], L)
            lags.append(np.ascontiguousarray(L.T))
        wlagT = np.stack(lags)  # [NLAG, T, T]
    return wblkT.astype(np.float32), idx, wlagT.astype(np.float32)


_WBLKT, _BLKIDX, _WLAGT = _host_consts()
NU = _WBLKT.shape[0]

_NC_CACHE = {}


def build_nc():
    key = 0
    if key in _NC_CACHE:
        return _NC_CACHE[key]
    nc = bacc.Bacc()

    x_d = nc.declare_dram_parameter("x", [S, BL, D], F32, isOutput=False)
    wp_d = nc.declare_dram_parameter("wp", [D, D], BF16, isOutput=False)
    zb_d = nc.declare_dram_parameter("zb", [D], F32, isOutput=False)
    w1t_d = nc.declare_dram_parameter("w1t", [D, FF], F8, isOutput=False)
    hb_d = nc.declare_dram_parameter("hb", [FF], F32, isOutput=False)
    w2t_d = nc.declare_dram_parameter("w2t", [FF, D], F8, isOutput=False)
    b2_d = nc.declare_dram_parameter("b2", [D], F32, isOutput=False)
    wblk_d = nc.declare_dram_parameter("wblk", [NU, T, T], F32, isOutput=False)
    wlag_d = nc.declare_dram_parameter("wlag", [NLAG, T, T], F32, isOutput=False)
    out_d = nc.declare_dram_parameter("out", [S, BL, D], F32, isOutput=True)

    with tile.TileContext(nc) as tc, ExitStack() as ctx:
        pool = lambda name, bufs, **kw: ctx.enter_context(
            tc.tile_pool(name=name, bufs=bufs, **kw)
        )
        wgt = pool("wgt", 1)
        stage = pool("stage", 1)
        xin = pool("xin", 9)
        lnp = pool("ln", 4)
        yppp = pool("ypp", 3)
        xtp = pool("xt", 6)
        y2tp = pool("y2t", 2)
        zp = pool("z", 10)
        x2p = pool("x2", 8)
        hp = pool("h", 2)
        outp = pool("outp", 3)
        # single full-bank PSUM rotation; transposes write through bitcast
        # views so bf16/fp8 tiles share the same bank pool
        psmm = pool("psmm", 8, space="PSUM")

        # ---------------- one-time setup ----------------
        xpre = {}

        def preload_x(i):
            b, c = steps[i]
            tiles = []
            for t in range(CB):
                s0 = (c * CB + t) * T
                xt = xin.tile([128, D], F32, tag="x")
                nc.sync.dma_start(xt[:], x_d.ap()[s0 : s0 + T, b, :])
                tiles.append(xt)
            xpre[i] = tiles

        steps = [(b, c) for b in range(BL) for c in range(NCHUNK)]
        preload_x(0)
        ident_f = stage.tile([128, 128], F32, tag="ident_f")
        make_identity(nc, ident_f[:])
        identB = wgt.tile([128, 128], BF16, tag="identB")
        nc.vector.tensor_copy(identB[:], ident_f[:])
        ident8 = wgt.tile([128, 128], F8, tag="ident8")
        nc.vector.tensor_copy(ident8[:], ident_f[:])
        eps_t = wgt.tile([128, 1], F32, tag="eps")
        nc.vector.memset(eps_t[:], EPS)
        zb_bc = wgt.tile([128, D], F32, tag="zb")
        nc.sync.dma_start(
            zb_bc[:], bass.AP(tensor=zb_d, offset=0, ap=[[0, 128], [1, D]])
        )
        b2_bc = wgt.tile([128, D], F32, tag="b2")
        nc.sync.dma_start(
            b2_bc[:], bass.AP(tensor=b2_d, offset=0, ap=[[0, 128], [1, D]])
        )
        b2b = wgt.tile([128, D], BF16, tag="b2b")
        nc.vector.tensor_copy(b2b[:], b2_bc[:])
        hb_sb = wgt.tile([128, NFT], F32, tag="hb")
        nc.sync.dma_start(
            hb_sb[:], bass.AP(tensor=hb_d, offset=0, ap=[[1, 128], [128, NFT]])
        )

        # mixing matrices: f32 DRAM -> resident f32r via casting DMA (SWDGE)
        wblk_r = wgt.tile([128, NU, T], F32R, tag="wblk")
        nc.gpsimd.dma_start(wblk_r[:], wblk_d.ap().rearrange("b j r -> j b r"))
        wlag_r = wgt.tile([128, NLAG, T], F32R, tag="wlag")
        nc.gpsimd.dma_start(wlag_r[:], wlag_d.ap().rearrange("b j r -> j b r"))

        # projection weight: bf16 straight from DRAM
        wp_r = wgt.tile([128, KD, D], BF16, tag="wp")
        nc.sync.dma_start(wp_r[:], wp_d.ap().rearrange("(kd p) e -> p kd e", p=128))

        # ---------------- helpers ----------------
        def ln_stats(xts, tag):
            """4 tiles -> (mv4 [128,4,2], r4 [128,4]) batched sqrt+recip."""
            mv4 = lnp.tile([128, CB, 2], F32, tag=f"mv4{tag}")
            for t in range(CB):
                st = lnp.tile([128, 6], F32, tag=f"bnst{tag}")
                nc.vector.bn_stats(st[:], xts[t][:])
                nc.vector.bn_aggr(mv4[:, t, :], st[:])
            r4 = lnp.tile([128, CB], F32, tag=f"r4{tag}")
            nc.scalar.activation(
                r4[:], mv4[:, :, 1], AF.Sqrt, bias=eps_t[:], scale=1.0
            )
            nc.vector.reciprocal(r4[:], r4[:])
            return mv4, r4

        # ---------------- main pipeline ----------------
        zall = {b: [] for b in range(BL)}
        a_out, b_out = {}, {}

        def stage_ln1(i):
            """LN1 + transpose: -> xT tiles (bf16, [d, kd, tok])."""
            b, c = steps[i]
            if i not in xpre:
                preload_x(i)
            xts = xpre.pop(i)
            mv4, r4 = ln_stats(xts, "a")
            xT = []
            for t in range(CB):
                ypp = yppp.tile([128, D], BF16, tag="ypp")
                nc.gpsimd.tensor_scalar(
                    out=ypp[:],
                    in0=xts[t][:],
                    scalar1=mv4[:, t, 0:1],
                    scalar2=r4[:, t : t + 1],
                    op0=ALU.subtract,
                    op1=ALU.mult,
                )
                ptb = psmm.tile([128, D], F32, tag="mm")
                pt = ptb[:].bitcast(BF16)  # [128, 1024] view, use cols 0..511
                for kd in range(KD):
                    nc.tensor.transpose(
                        pt[:, kd * 128 : (kd + 1) * 128],
                        ypp[:, kd * 128 : (kd + 1) * 128],
                        identB[:],
                    )
                xTt = xtp.tile([128, KD, 128], BF16, tag="xT")
                nc.vector.tensor_copy(
                    xTt[:], pt[:, 0:D].rearrange("p (a b) -> p a b", b=128)
                )
                xT.append(xTt)
            a_out[i] = (xts, xT)

        def stage_proj(i):
            _, xT = a_out[i]
            pzs = []
            for t in range(CB):
                pz = psmm.tile([128, D], F32, tag="mm")
                for kd in range(KD):
                    nc.tensor.matmul(
                        pz[:],
                        xT[t][:, kd, :],
                        wp_r[:, kd, :],
                        start=(kd == 0),
                        stop=(kd == KD - 1),
                    )
                pzs.append(pz)
            b_out[i] = pzs

        def stage_zev(i):
            b, c = steps[i]
            for t in range(CB):
                zt = zp.tile([128, D], F32R, tag="z")
                nc.vector.tensor_add(zt[:], b_out[i][t][:], zb_bc[:])
                zall[b].append(zt)
            del b_out[i]

        stage_ln1(0)
        stage_proj(0)
        stage_zev(0)

        # big fp8 weights: DMA'd in chunks interleaved with the early
        # pipeline so x loads aren't blocked behind the weight traffic.
        w1t_r = wgt.tile([128, KD, FF], F8, tag="w1t")
        w2t_r = wgt.tile([128, NFT, D], F8, tag="w2t")
        w1t_ap = w1t_d.ap().rearrange("(kd p) f -> p kd f", p=128)
        w2t_ap = w2t_d.ap().rearrange("(kf p) d -> p kf d", p=128)
        wload = [
            lambda kd2=kd2: nc.sync.dma_start(
                w1t_r[:, 2 * kd2 : 2 * kd2 + 2, :], w1t_ap[:, 2 * kd2 : 2 * kd2 + 2, :]
            )
            for kd2 in range(KD // 2)
        ] + [
            lambda f8=f8: nc.sync.dma_start(
                w2t_r[:, 8 * f8 : 8 * f8 + 8, :], w2t_ap[:, 8 * f8 : 8 * f8 + 8, :]
            )
            for f8 in range(2)
        ]
        wload.reverse()  # pop() from the front
        wload.pop()()

        def ffn1(y2T):
            """fp8 DoubleRow FFN1 + relu (split Act/DVE) -> ht."""
            ht = hp.tile([128, NFT, CB * 128], F8, tag="h")
            for ft in range(NFT):
                ph = psmm.tile([128, CB * 128], F32, tag="mm")
                for j in range(KD // 2):
                    nc.tensor.matmul(
                        ph[:],
                        w1t_r[:, 2 * j : 2 * j + 2, ft * 128 : (ft + 1) * 128],
                        y2T[:, 2 * j : 2 * j + 2, :],
                        start=(j == 0),
                        stop=(j == KD // 2 - 1),
                        perf_mode=DR,
                    )
                if ft in (5, 10, 15)[:N_RELU_DVE]:
                    nc.vector.tensor_scalar(
                        out=ht[:, ft, :],
                        in0=ph[:],
                        scalar1=hb_sb[:, ft : ft + 1],
                        scalar2=0.0,
                        op0=ALU.add,
                        op1=ALU.max,
                    )
                else:
                    nc.scalar.activation(
                        ht[:, ft, :],
                        ph[:],
                        AF.Relu,
                        bias=hb_sb[:, ft : ft + 1],
                        scale=1.0,
                    )
            return ht

        def ffn2(ht, x2pbs, b, c):
            """fp8 DoubleRow FFN2 + (x2+b2) via identity matmul -> out DMA."""
            for t in range(CB):
                s0 = (c * CB + t) * T
                po = psmm.tile([128, D], F32, tag="mm")
                for j in range(NFT // 2):
                    nc.tensor.matmul(
                        po[:],
                        ht[:, 2 * j : 2 * j + 2, t * 128 : (t + 1) * 128],
                        w2t_r[:, 2 * j : 2 * j + 2, :],
                        start=(j == 0),
                        stop=False,
                        perf_mode=DR,
                    )
                nc.tensor.matmul(
                    po[:], identB[:], x2pbs[t][:], start=False, stop=True
                )
                ot = outp.tile([128, D], F32, tag="o")
                nc.scalar.activation(ot[:], po[:], AF.Copy)
                nc.sync.dma_start(out_d.ap()[s0 : s0 + T, b, :], ot[:])

        # Software pipeline, one-step-deep FFN deferral: iteration i runs
        # step i's mixing/LN2 and step i+1's LN1/projection, with step i-1's
        # FFN matmuls emitted where the PE would otherwise stall on the LN
        # stat chains (DVE/Act/Pool latency).
        ffn_prev = None
        for i, (b, c) in enumerate(steps):
            xts, _ = a_out.pop(i)
            # --- mixing (banded decay matmul) ---
            pms = []
            for t in range(CB):
                blk = c * CB + t
                nmix = 1 + min(blk, NLAG)
                pm = psmm.tile([128, D], F32, tag="mm")
                nc.tensor.matmul(
                    pm[:],
                    wblk_r[:, _BLKIDX[blk], :],
                    zall[b][blk][:],
                    start=True,
                    stop=(nmix == 1),
                )
                for l in range(1, nmix):
                    nc.tensor.matmul(
                        pm[:],
                        wlag_r[:, l - 1, :],
                        zall[b][blk - l][:],
                        start=False,
                        stop=(l == nmix - 1),
                    )
                pms.append(pm)
            if i + 1 < len(steps):
                preload_x(i + 1)
            # --- x2 = x + attn (DVE, bf16); LN2 stats immediately after ---
            x2ts = []
            for t in range(CB):
                x2t = x2p.tile([128, D], BF16, tag="x2")
                nc.vector.tensor_add(x2t[:], pms[t][:], xts[t][:])
                x2ts.append(x2t)
            mv4b, r4b = ln_stats(x2ts, "b")
            x2pbs = []
            for t in range(CB):
                x2pb = x2p.tile([128, D], BF16, tag="x2pb")
                nc.gpsimd.tensor_add(x2pb[:], x2ts[t][:], b2b[:])
                x2pbs.append(x2pb)
            # --- FFN1 of the previous step: PE filler for the LN2 chain ---
            if ffn_prev is not None:
                ht_prev = ffn1(ffn_prev[0])
            # --- LN2 -> y2T (fp8, [d, kd, tok(chunk)]) ---
            y2T = y2tp.tile([128, KD, CB * 128], F8, tag="y2T")
            for t in range(CB):
                y2pp = yppp.tile([128, D], F8, tag="y2pp")
                nc.gpsimd.tensor_scalar(
                    out=y2pp[:],
                    in0=x2ts[t][:],
                    scalar1=mv4b[:, t, 0:1],
                    scalar2=r4b[:, t : t + 1],
                    op0=ALU.subtract,
                    op1=ALU.mult,
                )
                pt2b = psmm.tile([128, D], F32, tag="mm")
                pt2 = pt2b[:].bitcast(F8)  # [128, 2048] view, use cols 0..511
                for kd in range(KD):
                    nc.tensor.transpose(
                        pt2[:, kd * 128 : (kd + 1) * 128],
                        y2pp[:, kd * 128 : (kd + 1) * 128],
                        ident8[:],
                    )
                nc.scalar.activation(
                    y2T[:, :, t * 128 : (t + 1) * 128],
                    pt2[:, 0:D].rearrange("p (a b) -> p a b", b=128),
                    AF.Copy,
                )
            # --- next step's LN1 (DVE/Pool work overlapping FFN2 below) ---
            if i + 1 < len(steps):
                stage_ln1(i + 1)
            if wload:
                wload.pop()()
            # --- FFN2 of the previous step: PE filler for the LN1 chain ---
            if ffn_prev is not None:
                ffn2(ht_prev, ffn_prev[1], ffn_prev[2], ffn_prev[3])
            if i + 1 < len(steps):
                stage_proj(i + 1)
            if wload:
                wload.pop()()
            if i + 1 < len(steps):
                stage_zev(i + 1)
            if wload:
                wload.pop()()
            ffn_prev = (y2T, x2pbs, b, c)
        # epilogue: last step's FFN
        ht_prev = ffn1(ffn_prev[0])
        ffn2(ht_prev, ffn_prev[1], ffn_prev[2], ffn_prev[3])

    nc.compile()
    _NC_CACHE[key] = nc
    return nc


def _prep_inputs(x, w_lin, b_lin, w1, b1, w2, b2, g1, beta1, g2, beta2):
    f32 = np.float32
    wp = np.ascontiguousarray(w_lin.T * g1[:, None]).astype(NP_BF16)
    zb = (w_lin.astype(np.float64) @ beta1.astype(np.float64) + b_lin).astype(f32)
    w1t = np.ascontiguousarray(w1.T * g2[:, None]).astype(NP_F8)
    hb = (w1.astype(np.float64) @ beta2.astype(np.float64) + b1).astype(f32)
    w2t = np.ascontiguousarray(w2.T).astype(NP_F8)
    shared = {
        "wp": wp,
        "zb": zb,
        "w1t": w1t,
        "hb": hb,
        "w2t": w2t,
        "b2": b2.astype(f32),
        "wblk": _WBLKT,
        "wlag": _WLAGT,
    }
    in_maps = []
    for cc in range(NCORES):
        m = dict(shared)
        m["x"] = np.ascontiguousarray(x[:, cc * BL : (cc + 1) * BL, :]).astype(f32)
        in_maps.append(m)
    return in_maps


def kernel(**inputs):
    nc = build_nc()
    in_maps = _prep_inputs(**inputs)
    res = run_bass_kernel_spmd(nc, in_maps, list(range(NCORES)))
    out = np.concatenate([r["out"] for r in res.results], axis=1)
    return out.astype(np.float32)


if __name__ == "__main__":
    rng = np.random.default_rng(0)
    demo = {
        "x": rng.standard_normal((S, B, D)).astype(np.float32),
        "w_lin": rng.standard_normal((D, D)).astype(np.float32) * D**-0.5,
        "b_lin": rng.standard_normal((D,)).astype(np.float32) * 0.01,
        "w1": rng.standard_normal((FF, D)).astype(np.float32) * D**-0.5,
        "b1": rng.standard_normal((FF,)).astype(np.float32) * 0.01,
        "w2": rng.standard_normal((D, FF)).astype(np.float32) * FF**-0.5,
        "b2": rng.standard_normal((D,)).astype(np.float32) * 0.01,
        "g1": np.ones(D, np.float32),
        "beta1": np.zeros(D, np.float32),
        "g2": np.ones(D, np.float32),
        "beta2": np.zeros(D, np.float32),
    }
    out = kernel(**demo)
    print("ok", out.shape, out.dtype)


# revision 36
# speedup vs baseline: 1.9001x; 1.1079x over previous
"""Trainium2 Bass kernel for nn_ExpSelfAttention (dense transformer block).

Math (per batch item b, all f32 data):
    y  = LN(x; g1, beta1);  z = y @ w_lin.T + b_lin
    attn = W @ z            (W = causal exp-decay matrix, alpha=0.9)
    x2 = x + attn
    y2 = LN(x2; g2, beta2); h = relu(y2 @ w1.T + b1)
    out = x2 + h @ w2.T + b2

Sharding: data parallel over batch (16 / 8 cores = 2 per core); weights and
the (input-independent) decay-matrix blocks replicated. No collectives.

Precision plan (rel-err budget 2e-2, this lands ~5e-3):
  - FFN matmuls in fp8-e4m3 with MatmulPerfMode.DoubleRow (packs two
    contraction rows per PE cell: 0.5 cyc/output-row and K=256 per
    instruction -> 4x the f32r FLOP rate). y2/h/w1/w2 quantized to fp8.
  - Projection in bf16 (feeds the decay mixing whose output dominates the
    result -- fp8 there would blow the error budget); mixing in f32r.
  - x2 residual held in bf16; final output assembled in f32.

Engine balance (per-batch-item busy, approx): PE 55us (proj 14, mix 7,
FFN 27, transposes 7), DVE 53us (bn_stats/aggr, z+x2 PSUM evict-adds,
xT evict copies, 3/16 of the relus), Act 50us (relu 13/16, y2T evicts,
final out copies, sqrt), Pool/gpsimd 44us (both LN normalizes, x2+b2).
b2 and x2 are folded into the FFN2 PSUM accumulation via an identity-
weight matmul so the output eviction is a pure Act copy.

All big weights are pre-cast on the host and passed as fp8/bf16 DRAM
parameters (halves weight DMA traffic; no on-chip cast pass).
"""

import sys
from contextlib import ExitStack

for _p in ("/opt/trn_rl_repo", "/opt/pypackages"):
    if _p not in sys.path:
        sys.path.insert(0, _p)

import numpy as np
import ml_dtypes

import concourse.bass as bass
import concourse.mybir as mybir
import concourse.tile as tile
from concourse import bacc
from concourse.bass_utils import run_bass_kernel_spmd
from concourse.masks import make_identity

ALPHA, EPS = 0.9, 1e-5
S, B, D, FF = 2048, 16, 512, 2048
NCORES = 8
BL = B // NCORES            # batch items per core
T = 128                     # token tile
CB = 4                      # token tiles per chunk
NBLK = S // T               # 16
NCHUNK = NBLK // CB         # 4
NFT = FF // 128             # 16 f-tiles
KD = D // 128               # 4 d-tiles
NLAG = 1                    # decay lag blocks kept (lag>=2 < 2e-12 relative)
DVE_RELUS = (7, 15)         # these relu f-tiles run on DVE, rest on Act

F32 = mybir.dt.float32
F32R = mybir.dt.float32r
BF16 = mybir.dt.bfloat16
F8 = mybir.dt.float8e4
AF = mybir.ActivationFunctionType
ALU = mybir.AluOpType
DR = mybir.MatmulPerfMode.DoubleRow

NP_F8 = ml_dtypes.float8_e4m3
NP_BF16 = ml_dtypes.bfloat16


def _host_consts():
    """Decay-matrix derived constants, f64 -> f32 (mirrors reference)."""
    i = np.arange(S, dtype=np.float64)
    diff = i[:, None] - i[None, :]
    with np.errstate(under="ignore"):
        W = np.where(diff >= 0, ALPHA ** (diff + 1), 0.0)
        W = W + np.diag(1.0 - W.sum(axis=1))
        W = W.astype(np.float32)
        blocks = [
            np.ascontiguousarray(W[c * T : (c + 1) * T, c * T : (c + 1) * T].T)
            for c in range(NBLK)
        ]
        uniq, idx = [], []
        for blk in blocks:
            for j, u in enumerate(uniq):
                if np.array_equal(blk, u):
                    idx.append(j)
                    break
            else:
                idx.append(len(uniq))
                uniq.append(blk)
        wblkT = np.stack(uniq)  # [NU, T, T]
        lags = []
        for l in range(1, NLAG + 1):
            L = W[l * T : (l + 1) * T, 0:T]
            for i0 in range(l * T, S, T):
                assert np.array_equal(W[i0 : i0 + T, i0 - l * T : i0 - (l - 1) * T], L)
            lags.append(np.ascontiguousarray(L.T))
        wlagT = np.stack(lags)  # [NLAG, T, T]
    return wblkT.astype(np.float32), idx, wlagT.astype(np.float32)


_WBLKT, _BLKIDX, _WLAGT = _host_consts()
NU = _WBLKT.shape[0]

_NC_CACHE = {}


def build_nc():
    key = 0
    if key in _NC_CACHE:
        return _NC_CACHE[key]
    nc = bacc.Bacc()

    x_d = nc.declare_dram_parameter("x", [S, BL, D], F32, isOutput=False)
    wp_d = nc.declare_dram_parameter("wp", [D, D], BF16, isOutput=False)
    zb_d = nc.declare_dram_parameter("zb", [D], F32, isOutput=False)
    w1t_d = nc.declare_dram_parameter("w1t", [D, FF], F8, isOutput=False)
    hb_d = nc.declare_dram_parameter("hb", [FF], F32, isOutput=False)
    w2t_d = nc.declare_dram_parameter("w2t", [FF, D], F8, isOutput=False)
    b2_d = nc.declare_dram_parameter("b2", [D], F32, isOutput=False)
    wblk_d = nc.declare_dram_parameter("wblk", [NU, T, T], F32, isOutput=False)
    wlag_d = nc.declare_dram_parameter("wlag", [NLAG, T, T], F32, isOutput=False)
    out_d = nc.declare_dram_parameter("out", [S, BL, D], F32, isOutput=True)

    with tile.TileContext(nc) as tc, ExitStack() as ctx:
        pool = lambda name, bufs, **kw: ctx.enter_context(
            tc.tile_pool(name=name, bufs=bufs, **kw)
        )
        wgt = pool("wgt", 1)
        stage = pool("stage", 1)
        xin = pool("xin", 9)
        lnp = pool("ln", 4)
        yppp = pool("ypp", 3)
        xtp = pool("xt", 6)
        y2tp = pool("y2t", 2)
        zp = pool("z", 10)
        x2p = pool("x2", 8)
        hp = pool("h", 2)
        outp = pool("outp", 3)
        # single full-bank PSUM rotation; transposes write through bitcast
        # views so bf16/fp8 tiles share the same bank pool
        psmm = pool("psmm", 8, space="PSUM")

        # ---------------- one-time setup ----------------
        xpre = {}

        def preload_x(i):
            b, c = steps[i]
            tiles = []
            for t in range(CB):
                s0 = (c * CB + t) * T
                xt = xin.tile([128, D], F32, tag="x")
                nc.sync.dma_start(xt[:], x_d.ap()[s0 : s0 + T, b, :])
                tiles.append(xt)
            xpre[i] = tiles

        # DMA order matters: the shared DMA device drains FIFO, so x(0) and
        # the weights the first iteration blocks on (zb, wp, wblk) go first.
        steps = [(b, c) for b in range(BL) for c in range(NCHUNK)]
        preload_x(0)
        zb_bc = wgt.tile([128, D], F32, tag="zb")
        nc.sync.dma_start(
            zb_bc[:], bass.AP(tensor=zb_d, offset=0, ap=[[0, 128], [1, D]])
        )
        # projection weight: bf16 straight from DRAM
        wp_r = wgt.tile([128, KD, D], BF16, tag="wp")
        nc.sync.dma_start(wp_r[:], wp_d.ap().rearrange("(kd p) e -> p kd e", p=128))
        # mixing matrices: f32 DRAM -> resident f32r via casting DMA (SWDGE)
        wblk_r = wgt.tile([128, NU, T], F32R, tag="wblk")
        nc.gpsimd.dma_start(wblk_r[:], wblk_d.ap().rearrange("b j r -> j b r"))
        wlag_r = wgt.tile([128, NLAG, T], F32R, tag="wlag")
        nc.gpsimd.dma_start(wlag_r[:], wlag_d.ap().rearrange("b j r -> j b r"))
        preload_x(1)
        b2_bc = wgt.tile([128, D], F32, tag="b2")
        nc.sync.dma_start(
            b2_bc[:], bass.AP(tensor=b2_d, offset=0, ap=[[0, 128], [1, D]])
        )
        hb_sb = wgt.tile([128, NFT], F32, tag="hb")
        nc.sync.dma_start(
            hb_sb[:], bass.AP(tensor=hb_d, offset=0, ap=[[1, 128], [128, NFT]])
        )
        ident_f = stage.tile([128, 128], F32, tag="ident_f")
        make_identity(nc, ident_f[:])
        identB = wgt.tile([128, 128], BF16, tag="identB")
        nc.vector.tensor_copy(identB[:], ident_f[:])
        ident8 = wgt.tile([128, 128], F8, tag="ident8")
        nc.vector.tensor_copy(ident8[:], ident_f[:])
        eps_t = wgt.tile([128, 1], F32, tag="eps")
        nc.vector.memset(eps_t[:], EPS)
        # tiny dummy activation: triggers the one-time activation-table load
        # while the pipeline is still waiting on the first x DMAs
        warm_t = wgt.tile([128, 1], F32, tag="warm")
        nc.scalar.activation(warm_t[:], eps_t[:], AF.Sqrt, bias=eps_t[:], scale=1.0)
        b2b = wgt.tile([128, D], BF16, tag="b2b")
        nc.vector.tensor_copy(b2b[:], b2_bc[:])

        # ---------------- helpers ----------------
        def ln_stats(xts, tag, per_tile=False):
            """4 tiles -> (mv4 [128,4,2], r4 [128,4]).

            batched (default): one sqrt+recip over all 4 tiles (fewer Act
            instrs); per_tile: sqrt/recip per tile so tile 0's normalize can
            start before tile 3's stats land (pipeline fill/drain).
            """
            mv4 = lnp.tile([128, CB, 2], F32, tag=f"mv4{tag}")
            r4 = lnp.tile([128, CB], F32, tag=f"r4{tag}")
            for t in range(CB):
                st = lnp.tile([128, 6], F32, tag=f"bnst{tag}")
                nc.vector.bn_stats(st[:], xts[t][:])
                nc.vector.bn_aggr(mv4[:, t, :], st[:])
                if per_tile:
                    nc.scalar.activation(
                        r4[:, t : t + 1], mv4[:, t, 1:2], AF.Sqrt,
                        bias=eps_t[:], scale=1.0,
                    )
                    nc.vector.reciprocal(r4[:, t : t + 1], r4[:, t : t + 1])
            if not per_tile:
                nc.scalar.activation(
                    r4[:], mv4[:, :, 1], AF.Sqrt, bias=eps_t[:], scale=1.0
                )
                nc.vector.reciprocal(r4[:], r4[:])
            return mv4, r4

        # ---------------- main pipeline ----------------
        zall = {b: [] for b in range(BL)}
        a_out, b_out = {}, {}

        def stage_ln1(i, per_tile=False, interleave_zev=False):
            """LN1 + transpose + projection, tile-interleaved on the PE."""
            b, c = steps[i]
            if i not in xpre:
                preload_x(i)
            xts = xpre.pop(i)
            mv4, r4 = ln_stats(xts, "a", per_tile=per_tile)
            xT = []
            pzs = []

            def proj_tile(t):
                pz = psmm.tile([128, D], F32, tag="mm")
                for kd in range(KD):
                    nc.tensor.matmul(
                        pz[:],
                        xT[t][:, kd, :],
                        wp_r[:, kd, :],
                        start=(kd == 0),
                        stop=(kd == KD - 1),
                    )
                pzs.append(pz)
                if interleave_zev:
                    zt = zp.tile([128, D], F32R, tag="z")
                    nc.vector.tensor_add(zt[:], pz[:], zb_bc[:])
                    zall[b].append(zt)

            # one-tile lag: proj(t-1) is emitted after transpose(t), so the
            # PE runs proj(t-1) while tile t's evict copy (DVE) completes
            for t in range(CB):
                ypp = yppp.tile([128, D], BF16, tag="ypp")
                nc.gpsimd.tensor_scalar(
                    out=ypp[:],
                    in0=xts[t][:],
                    scalar1=mv4[:, t, 0:1],
                    scalar2=r4[:, t : t + 1],
                    op0=ALU.subtract,
                    op1=ALU.mult,
                )
                ptb = psmm.tile([128, D], F32, tag="mm")
                pt = ptb[:].bitcast(BF16)  # [128, 1024] view, use cols 0..511
                for kd in range(KD):
                    nc.tensor.transpose(
                        pt[:, kd * 128 : (kd + 1) * 128],
                        ypp[:, kd * 128 : (kd + 1) * 128],
                        identB[:],
                    )
                xTt = xtp.tile([128, KD, 128], BF16, tag="xT")
                nc.vector.tensor_copy(
                    xTt[:], pt[:, 0:D].rearrange("p (a b) -> p a b", b=128)
                )
                xT.append(xTt)
                if t >= 1:
                    proj_tile(t - 1)
            proj_tile(CB - 1)
            a_out[i] = (xts, xT)
            b_out[i] = pzs

        def stage_zev(i):
            b, c = steps[i]
            for t in range(CB):
                zt = zp.tile([128, D], F32R, tag="z")
                nc.vector.tensor_add(zt[:], b_out[i][t][:], zb_bc[:])
                zall[b].append(zt)
            del b_out[i]

        stage_ln1(0, per_tile=True, interleave_zev=True)
        del b_out[0]

        # big fp8 weights: DMA'd in chunks interleaved with the early
        # pipeline so x loads aren't blocked behind the weight traffic.
        w1t_r = wgt.tile([128, KD, FF], F8, tag="w1t")
        w2t_r = wgt.tile([128, NFT, D], F8, tag="w2t")
        w1t_ap = w1t_d.ap().rearrange("(kd p) f -> p kd f", p=128)
        w2t_ap = w2t_d.ap().rearrange("(kf p) d -> p kf d", p=128)
        wload = [
            lambda kd2=kd2: nc.sync.dma_start(
                w1t_r[:, 2 * kd2 : 2 * kd2 + 2, :], w1t_ap[:, 2 * kd2 : 2 * kd2 + 2, :]
            )
            for kd2 in range(KD // 2)
        ] + [
            lambda f8=f8: nc.sync.dma_start(
                w2t_r[:, 8 * f8 : 8 * f8 + 8, :], w2t_ap[:, 8 * f8 : 8 * f8 + 8, :]
            )
            for f8 in range(2)
        ]
        wload.reverse()  # pop() from the front
        wload.pop()()

        def ffn1_part(y2T, ht, fts, dve_relus=DVE_RELUS):
            """fp8 DoubleRow FFN1 + relu (split Act/DVE) for given f-tiles."""
            for ft in fts:
                ph = psmm.tile([128, CB * 128], F32, tag="mm")
                for j in range(KD // 2):
                    nc.tensor.matmul(
                        ph[:],
                        w1t_r[:, 2 * j : 2 * j + 2, ft * 128 : (ft + 1) * 128],
                        y2T[:, 2 * j : 2 * j + 2, :],
                        start=(j == 0),
                        stop=(j == KD // 2 - 1),
                        perf_mode=DR,
                    )
                if ft in dve_relus:
                    nc.vector.tensor_scalar(
                        out=ht[:, ft, :],
                        in0=ph[:],
                        scalar1=hb_sb[:, ft : ft + 1],
                        scalar2=0.0,
                        op0=ALU.add,
                        op1=ALU.max,
                    )
                else:
                    nc.scalar.activation(
                        ht[:, ft, :],
                        ph[:],
                        AF.Relu,
                        bias=hb_sb[:, ft : ft + 1],
                        scale=1.0,
                    )

        def ffn2_part(ht, x2pbs, b, c, ts):
            """fp8 DoubleRow FFN2 + (x2+b2) via identity matmul -> out DMA."""
            for t in ts:
                s0 = (c * CB + t) * T
                po = psmm.tile([128, D], F32, tag="mm")
                for j in range(NFT // 2):
                    nc.tensor.matmul(
                        po[:],
                        ht[:, 2 * j : 2 * j + 2, t * 128 : (t + 1) * 128],
                        w2t_r[:, 2 * j : 2 * j + 2, :],
                        start=(j == 0),
                        stop=False,
                        perf_mode=DR,
                    )
                nc.tensor.matmul(
                    po[:], identB[:], x2pbs[t][:], start=False, stop=True
                )
                ot = outp.tile([128, D], F32, tag="o")
                nc.scalar.activation(ot[:], po[:], AF.Copy)
                nc.sync.dma_start(out_d.ap()[s0 : s0 + T, b, :], ot[:])

        # Software pipeline, one-step-deep FFN deferral: iteration i runs
        # step i's mixing/LN2 and step i+1's LN1/projection, with step i-1's
        # FFN matmuls emitted where the PE would otherwise stall on the LN
        # stat chains (DVE/Act/Pool latency).
        ffn_prev = None
        for i, (b, c) in enumerate(steps):
            xts, _ = a_out.pop(i)
            # --- mixing (banded decay matmul) ---
            pms = []
            for t in range(CB):
                blk = c * CB + t
                nmix = 1 + min(blk, NLAG)
                pm = psmm.tile([128, D], F32, tag="mm")
                nc.tensor.matmul(
                    pm[:],
                    wblk_r[:, _BLKIDX[blk], :],
                    zall[b][blk][:],
                    start=True,
                    stop=(nmix == 1),
                )
                for l in range(1, nmix):
                    nc.tensor.matmul(
                        pm[:],
                        wlag_r[:, l - 1, :],
                        zall[b][blk - l][:],
                        start=False,
                        stop=(l == nmix - 1),
                    )
                pms.append(pm)
            if i + 1 < len(steps):
                preload_x(i + 1)
            # --- x2 = x + attn (DVE, bf16) with tile-interleaved LN2 stats
            # (per-tile sqrt/recip so norm2(t0) starts ~4us earlier than a
            # batched chain would allow) ---
            x2ts = []
            mv4b = lnp.tile([128, CB, 2], F32, tag="mv4b")
            r4b = lnp.tile([128, CB], F32, tag="r4b")
            for t in range(CB):
                x2t = x2p.tile([128, D], BF16, tag="x2")
                nc.vector.tensor_add(x2t[:], pms[t][:], xts[t][:])
                x2ts.append(x2t)
                st = lnp.tile([128, 6], F32, tag="bnstb")
                nc.vector.bn_stats(st[:], x2t[:])
                nc.vector.bn_aggr(mv4b[:, t, :], st[:])
                nc.scalar.activation(
                    r4b[:, t : t + 1], mv4b[:, t, 1:2], AF.Sqrt,
                    bias=eps_t[:], scale=1.0,
                )
                nc.vector.reciprocal(r4b[:, t : t + 1], r4b[:, t : t + 1])
            # --- LN2 per-tile emitter (norm2 Pool, T2 PE, evict Act/DVE) ---
            y2T = y2tp.tile([128, KD, CB * 128], F8, tag="y2T")

            def ln2_tile(t):
                y2pp = yppp.tile([128, D], F8, tag="y2pp")
                nc.gpsimd.tensor_scalar(
                    out=y2pp[:],
                    in0=x2ts[t][:],
                    scalar1=mv4b[:, t, 0:1],
                    scalar2=r4b[:, t : t + 1],
                    op0=ALU.subtract,
                    op1=ALU.mult,
                )
                pt2b = psmm.tile([128, D], F32, tag="mm")
                pt2 = pt2b[:].bitcast(F8)  # [128, 2048] view, use cols 0..511
                for kd in range(KD):
                    nc.tensor.transpose(
                        pt2[:, kd * 128 : (kd + 1) * 128],
                        y2pp[:, kd * 128 : (kd + 1) * 128],
                        ident8[:],
                    )
                if i + 1 == len(steps):
                    # last iteration: Act is clogged with this iteration's
                    # relus/outs -- evict on (idle) DVE so the epilogue FFN
                    # isn't stalled behind them
                    nc.vector.tensor_copy(
                        y2T[:, :, t * 128 : (t + 1) * 128],
                        pt2[:, 0:D].rearrange("p (a b) -> p a b", b=128),
                    )
                else:
                    nc.scalar.activation(
                        y2T[:, :, t * 128 : (t + 1) * 128],
                        pt2[:, 0:D].rearrange("p (a b) -> p a b", b=128),
                        AF.Copy,
                    )

            # --- previous step's FFN: chunky ready-to-run PE work covering
            # this step's LN2 chain and next step's LN1 chain ---
            if ffn_prev is not None:
                ht_prev = hp.tile([128, NFT, CB * 128], F8, tag="h")
                ffn1_part(ffn_prev[0], ht_prev, range(NFT))
                ffn2_part(ht_prev, ffn_prev[1], ffn_prev[2], ffn_prev[3], (0, 1, 2, 3))
            ln2_tile(0)
            ln2_tile(1)
            ln2_tile(2)
            ln2_tile(3)
            if wload:
                wload.pop()()
            # --- next step's LN1 ---
            if i + 1 < len(steps):
                stage_ln1(i + 1)
            # x2+b2 (Pool) late: only needed by next iteration's ffn2
            x2pbs = []
            for t in range(CB):
                x2pb = x2p.tile([128, D], BF16, tag="x2pb")
                nc.gpsimd.tensor_add(x2pb[:], x2ts[t][:], b2b[:])
                x2pbs.append(x2pb)
            if wload:
                wload.pop()()
            if i + 1 < len(steps):
                stage_zev(i + 1)
            if wload:
                wload.pop()()
            ffn_prev = (y2T, x2pbs, b, c)
        # epilogue: last step's FFN; relu split 50/50 so neither engine's
        # queue becomes the drain tail
        ht_prev = hp.tile([128, NFT, CB * 128], F8, tag="h")
        ffn1_part(ffn_prev[0], ht_prev, range(NFT),
                  dve_relus=(1, 3, 5, 7, 9, 11, 13, 15))
        ffn2_part(ht_prev, ffn_prev[1], ffn_prev[2], ffn_prev[3], range(CB))

    nc.compile()
    _NC_CACHE[key] = nc
    return nc


def _prep_inputs(x, w_lin, b_lin, w1, b1, w2, b2, g1, beta1, g2, beta2):
    f32 = np.float32
    wp = np.ascontiguousarray(w_lin.T * g1[:, None]).astype(NP_BF16)
    zb = (w_lin.astype(np.float64) @ beta1.astype(np.float64) + b_lin).astype(f32)
    w1t = np.ascontiguousarray(w1.T * g2[:, None]).astype(NP_F8)
    hb = (w1.astype(np.float64) @ beta2.astype(np.float64) + b1).astype(f32)
    w2t = np.ascontiguousarray(w2.T).astype(NP_F8)
    shared = {
        "wp": wp,
        "zb": zb,
        "w1t": w1t,
        "hb": hb,
        "w2t": w2t,
        "b2": b2.astype(f32),
        "wblk": _WBLKT,
        "wlag": _WLAGT,
    }
    in_maps = []
    for cc in range(NCORES):
        m = dict(shared)
        m["x"] = np.ascontiguousarray(x[:, cc * BL : (cc + 1) * BL, :]).astype(f32)
        in_maps.append(m)
    return in_maps


def kernel(**inputs):
    nc = build_nc()
    in_maps = _prep_inputs(**inputs)
    res = run_bass_kernel_spmd(nc, in_maps, list(range(NCORES)))
    out = np.concatenate([r["out"] for r in res.results], axis=1)
    return out.astype(np.float32)


if __name__ == "__main__":
    rng = np.random.default_rng(0)
    demo = {
        "x": rng.standard_normal((S, B, D)).astype(np.float32),
        "w_lin": rng.standard_normal((D, D)).astype(np.float32) * D**-0.5,
        "b_lin": rng.standard_normal((D,)).astype(np.float32) * 0.01,
        "w1": rng.standard_normal((FF, D)).astype(np.float32) * D**-0.5,
        "b1": rng.standard_normal((FF,)).astype(np.float32) * 0.01,
        "w2": rng.standard_normal((D, FF)).astype(np.float32) * FF**-0.5,
        "b2": rng.standard_normal((D,)).astype(np.float32) * 0.01,
        "g1": np.ones(D, np.float32),
        "beta1": np.zeros(D, np.float32),
        "g2": np.ones(D, np.float32),
        "beta2": np.zeros(D, np.float32),
    }
    out = kernel(**demo)
    print("ok", out.shape, out.dtype)
